# revision 1
# baseline (speedup 1.0000x reference)
"""Causal self-attention with RoPE for trn2, 8-core head-parallel Bass kernel.

Problem (hardcoded): B=1, S=4096, D=1024, H=16 heads, head_dim=64, fp32.
  q/k/v = shape_heads(x @ W{q,k,v}.T); RoPE(q, k); causal softmax(q k^T / 8) v;
  out = concat_heads @ Wo.T

Sharding: 2 heads per core (column-parallel Wq/Wk/Wv, row-parallel Wo).
Each core computes a full-shape partial output; host sums the 8 partials.

Per-core device kernel (all matmuls fp32r = full-rate ~1e-4 precision):
  - x is fed pre-transposed (host) as xt (D, S); per 512-col slice:
      QT/KT/VT (128=2*64 head-rows, 512) = wT.T @ xt-slice    [8 K-tile matmuls]
      RoPE on QT/KT via signed-permutation matmul (rot = R @ q) + 3 DVE ops
      V natural (s,64) blocks via PE transpose of VT, ones column appended
  - attention in transposed layout: scoresT (k-part, q-free) so softmax
    denominators ride the PV matmul as an extra lhsT ones-row (M=65):
      scT = KT_h.T @ QT_h (two heads packed via PE row-tiling), exp on ACT
      (scale=1/8 folded in), causal mask = DVE mult on diagonal-band blocks,
      outT (65, 512) += [V|1].T @ PT   (row 64 = softmax denominator)
  - normalize: recip on DVE, broadcast via ones(1,64) matmul, one DVE mult
    into attnT; Wo row-parallel: out = attnT.T @ WoT per (128,512) tile,
    evicted PSUM->SBUF on DVE and DMA'd to DRAM.

Projections/RoPE are software-pipelined one q-slice ahead of attention.

Measured (8 axon trn2 cores): rel err 1.96e-4 vs fp32 reference; cost-model
timeline 251 us/core; HW slope benchmark (For_i x129) ~300 us/iter (+/-60us
measurement noise from the axon roundtrip baseline).
"""

import math
import numpy as np

import concourse.bass as bass
import concourse.mybir as mybir
import concourse.tile as tile
from concourse import bacc
from concourse.bass import ts
from concourse.bass_utils import run_bass_kernel_spmd
from concourse.masks import make_identity

F32 = mybir.dt.float32
F32R = mybir.dt.float32r
AF = mybir.ActivationFunctionType

S = 4096
D = 1024
HD = 64
N_CORES = 8
SCALE = 1.0 / math.sqrt(HD)
ROPE_BASE = 10000.0

import os
# tuned via TimelineSim sweeps + HW slope benchmarks; env-overridable for tests
MASK_MODE = os.environ.get("MASK_MODE", "dve")
WO_MODE = os.environ.get("WO_MODE", "tail")
BCAST_MODE = os.environ.get("BCAST_MODE", "pe")
PT_BUFS = int(os.environ.get("PT_BUFS", 3))
RAW_BUFS = int(os.environ.get("RAW_BUFS", 4))
TMP_BUFS = int(os.environ.get("TMP_BUFS", 4))
XT_BUFS = int(os.environ.get("XT_BUFS", 2))
KREPEAT = int(os.environ.get("KREPEAT", 1))   # >1 wraps body in For_i (benchmarking)
RAWS_ENG = os.environ.get("RAWS_ENG", "act")
MM_BUFS = int(os.environ.get("MM_BUFS", 2))
PV_BUFS = int(os.environ.get("PV_BUFS", 2))

NSL = S // 512    # 8 q-slices of 512
NT = D // 128     # 8 contraction tiles
NB = S // 128     # 32 k-blocks of 128


def _emit_wo(nc, mmps, tmpp, at_sb, wot_sb, out_d, trange):
    for t in trange:
        for n in range(2):
            wo = mmps.tile([128, 512], F32, tag="proj", name=f"wo_{t}_{n}")
            nc.tensor.matmul(wo[:], at_sb[:, ts(t, 128)],
                             wot_sb[:, ts(n, 512)], start=True, stop=True)
            wos = tmpp.tile([128, 512], F32, tag="wos", name=f"wos_{t}_{n}")
            if os.environ.get("WO_COPY", "dve") == "alt" and (t + n) % 2:
                nc.scalar.copy(wos[:], wo[:])
            else:
                nc.vector.tensor_copy(wos[:], wo[:])
            nc.sync.dma_start(out_d[ts(t, 128), ts(n, 512)], wos[:])


def _emit(tc):
    nc = tc.nc
    xt_d = nc.dram_tensor("xt", [D, S], F32, kind="ExternalInput").ap()
    wqt_d = nc.dram_tensor("wqt", [D, 128], F32, kind="ExternalInput").ap()
    wkt_d = nc.dram_tensor("wkt", [D, 128], F32, kind="ExternalInput").ap()
    wvt_d = nc.dram_tensor("wvt", [D, 128], F32, kind="ExternalInput").ap()
    wot_d = nc.dram_tensor("wot", [128, D], F32, kind="ExternalInput").ap()
    cost_d = nc.dram_tensor("cost", [128, S], F32, kind="ExternalInput").ap()
    sint_d = nc.dram_tensor("sint", [128, S], F32, kind="ExternalInput").ap()
    rmt_d = nc.dram_tensor("rmt", [128, 128], F32, kind="ExternalInput").ap()
    msk_d = nc.dram_tensor("msk", [128, 4096], mybir.dt.bfloat16, kind="ExternalInput").ap()
    ones_d = nc.dram_tensor("ones", [128, 64], F32, kind="ExternalInput").ap()
    out_d = nc.dram_tensor("out", [S, D], F32, kind="ExternalOutput").ap()

    import contextlib
    ctx = contextlib.ExitStack()
    with ctx:
        const = ctx.enter_context(tc.tile_pool(name="const", bufs=1))
        xtp = ctx.enter_context(tc.tile_pool(name="xtp", bufs=XT_BUFS))
        rawp = ctx.enter_context(tc.tile_pool(name="rawp", bufs=RAW_BUFS))
        tmpp = ctx.enter_context(tc.tile_pool(name="tmpp", bufs=TMP_BUFS))
        qkp = ctx.enter_context(tc.tile_pool(name="qkp", bufs=1))
        ptp = ctx.enter_context(tc.tile_pool(name="ptp", bufs=PT_BUFS))
        rcp = ctx.enter_context(tc.tile_pool(name="rcp", bufs=int(os.environ.get("RC_BUFS", 2))))
        atp = ctx.enter_context(tc.tile_pool(name="atp", bufs=1))
        mmps = ctx.enter_context(tc.tile_pool(name="mmps", bufs=MM_BUFS, space="PSUM"))
        scps = ctx.enter_context(tc.tile_pool(name="scps", bufs=2, space="PSUM"))
        pvps = ctx.enter_context(tc.tile_pool(name="pvps", bufs=PV_BUFS, space="PSUM"))

        # ---- constants ----
        # wq first, then the first xt slice on the SWDGE queue (parallel to
        # the HWDGE constant stream) so slice-0 matmuls start ASAP
        wq_sb = const.tile([128, D], F32R)
        wk_sb = const.tile([128, D], F32R)
        wv_sb = const.tile([128, D], F32R)
        nc.sync.dma_start(
            wq_sb[:].rearrange("p (t m) -> p t m", t=NT),
            wqt_d.rearrange("(t p) m -> p t m", p=128).bitcast(F32R))

        xt3 = xt_d.rearrange("(t p) s -> p t s", p=128).bitcast(F32R)
        xt_tiles = {}
        xt_tiles[0] = xtp.tile([128, NT * 512], F32R, tag="xt", name="xt_sb_0")
        nc.sync.dma_start(xt_tiles[0][:].rearrange("p (t s) -> p t s", t=NT),
                          xt3[:, :, ts(0, 512)])

        for w_sb, w_d in ((wk_sb, wkt_d), (wv_sb, wvt_d)):
            nc.sync.dma_start(
                w_sb[:].rearrange("p (t m) -> p t m", t=NT),
                w_d.rearrange("(t p) m -> p t m", p=128).bitcast(F32R),
            )
        rmt_sb = const.tile([128, 128], F32R)
        nc.sync.dma_start(rmt_sb[:], rmt_d.bitcast(F32R))
        ident = const.tile([128, 128], F32)
        make_identity(nc, ident[:])

        # warm up the ACT exp table early
        warm = const.tile([1, 16], F32)
        nc.vector.memset(warm[:], 0.0)
        nc.scalar.activation(warm[:], warm[:], AF.Exp)

        cost_sb = const.tile([128, S], F32)
        sint_sb = const.tile([128, S], F32)
        nc.sync.dma_start(cost_sb[:], cost_d)
        nc.sync.dma_start(sint_sb[:], sint_d)
        msk2_sb = const.tile([128, 4096], mybir.dt.bfloat16)
        nc.sync.dma_start(msk2_sb[:], msk_d)
        ones1_sb = const.tile([1, 64], F32R)
        nc.sync.dma_start(ones1_sb[:], ones_d[0:1, :].bitcast(F32R))
        wot_sb = const.tile([128, D], F32R)
        nc.sync.dma_start(wot_sb[:], wot_d.bitcast(F32R))

        # V natural storage: 32 blocks of (128, 130) = [V_h0 | 1 | V_h1 | 1]
        v_sb = const.tile([128, NB * 130], F32R, name="v_sb")
        v4 = v_sb[:].rearrange("p (b t c) -> p b t c", t=2, c=65)
        nc.sync.dma_start(v4[:, :, :, 64], ones_d.bitcast(F32R))

        qfin = qkp.tile([128, S], F32R)
        kfin = qkp.tile([128, S], F32R)
        at_sb = atp.tile([128, S], F32R)

        if KREPEAT > 1 and os.environ.get("KMODE", "unroll") == "for":
            with tc.For_i(0, KREPEAT, 1):
                _emit_body(tc, nc, locals())
        else:
            for _rep in range(KREPEAT):
                _emit_body(tc, nc, locals())


def _emit_body(tc, nc, env):
    (mmps, scps, pvps, xtp, rawp, tmpp, ptp, rcp, const,
     wq_sb, wk_sb, wv_sb, rmt_sb, ident, cost_sb, sint_sb, msk2_sb, ones1_sb,
     wot_sb, v_sb, v4, qfin, kfin, at_sb, xt_tiles, xt3, out_d) = (
        env["mmps"], env["scps"], env["pvps"], env["xtp"], env["rawp"],
        env["tmpp"], env["ptp"], env["rcp"], env["const"], env["wq_sb"],
        env["wk_sb"], env["wv_sb"], env["rmt_sb"], env["ident"],
        env["cost_sb"], env["sint_sb"], env["msk2_sb"], env["ones1_sb"],
        env["wot_sb"], env["v_sb"], env["v4"], env["qfin"], env["kfin"],
        env["at_sb"], env["xt_tiles"], env["xt3"], env["out_d"])
    if True:
        def emit_qkv(i):
            sl = ts(i, 512)
            # ---- load xt slice (slice 0 already in flight) ----
            if i not in xt_tiles:
                xt_tiles[i] = xtp.tile([128, NT * 512], F32R, tag="xt", name=f"xt_sb_{i}")
                nc.sync.dma_start(
                    xt_tiles[i][:].rearrange("p (t s) -> p t s", t=NT),
                    xt3[:, :, sl])
            xt_sb = xt_tiles[i]
            x3 = xt_sb[:].rearrange("p (t s) -> p t s", t=NT)

            # ---- projections ----
            raws = {}
            for nm, w_sb in (("q", wq_sb), ("k", wk_sb), ("v", wv_sb)):
                prj = mmps.tile([128, 512], F32, tag="proj", name=f"prj_{nm}_{i}")
                for t in range(NT):
                    nc.tensor.matmul(prj[:], w_sb[:, ts(t, 128)], x3[:, t, :],
                                     start=(t == 0), stop=(t == NT - 1))
                dt = F32 if nm == "v" else F32R
                raw = rawp.tile([128, 512], dt, tag="raw", name=f"raw_{nm}_{i}")
                nc.scalar.copy(raw[:], prj[:])
                raws[nm] = raw

            # ---- RoPE on q, k ----
            for nm, fin in (("q", qfin), ("k", kfin)):
                raw = raws[nm]
                rot = mmps.tile([128, 512], F32, tag="proj", name=f"rot_{nm}_{i}")
                nc.tensor.matmul(rot[:], rmt_sb[:], raw[:], start=True, stop=True)
                t1 = tmpp.tile([128, 512], F32, tag="tmp", name=f"t1_{nm}_{i}")
                nc.vector.tensor_mul(t1[:], rot[:], sint_sb[:, sl])
                t2 = tmpp.tile([128, 512], F32, tag="tmp", name=f"t2_{nm}_{i}")
                nc.vector.tensor_mul(t2[:], raw[:], cost_sb[:, sl])
                nc.vector.tensor_add(fin[:, sl], t1[:], t2[:])

            # ---- V natural blocks (both heads in one strided copy) ----
            for bi in range(4):
                b = 4 * i + bi
                vn = mmps.tile([128, 128], F32, tag="proj", name=f"vn_{b}")
                nc.tensor.transpose(vn[:], raws["v"][:, ts(bi, 128)], ident[:])
                if os.environ.get("VCOPY", "act") == "dve":
                    nc.vector.tensor_copy(v4[:, b, :, 0:64],
                                          vn[:].rearrange("p (t c) -> p t c", t=2))
                else:
                    nc.scalar.copy(v4[:, b, :, 0:64],
                                   vn[:].rearrange("p (t c) -> p t c", t=2))

        def emit_attn(i):
            sl = ts(i, 512)
            # ---- attention for this q-slice ----
            pvt = [pvps.tile([65, 512], F32, tag="pv", name=f"pv{h}_{i}")
                   for h in (0, 1)]
            nj = 4 * (i + 1)
            for j in range(nj):
                r = j - 4 * i          # >= 0 on diagonal-band blocks
                off = 128 * r if r >= 0 else 0
                w = 512 - off          # valid q-range of this k-block
                sc = scps.tile([128, 1024], F32, tag="sc", name=f"sc_{i}_{j}")
                for h in (0, 1):
                    hs = slice(64 * h, 64 * h + 64)
                    nc.tensor.matmul(sc[:, 512 * h + off:512 * h + 512],
                                     kfin[hs, ts(j, 128)],
                                     qfin[hs, 512 * i + off:512 * (i + 1)],
                                     start=True, stop=True,
                                     tile_position=(64 * h, 0))
                pt = ptp.tile([128, 1024], F32R, tag="pt", name=f"pt_{i}_{j}")
                if r < 0:
                    nc.scalar.activation(pt[:], sc[:], AF.Exp, scale=SCALE)
                else:
                    halves = lambda ap, base: ap[:, base:base + 1024].rearrange(
                        "p (t c) -> p t c", t=2)[:, :, off:512]
                    nc.scalar.activation(halves(pt, 0), halves(sc, 0),
                                         AF.Exp, scale=SCALE)
                    if MASK_MODE == "pool":
                        # keep iff q - k >= 0 with q = off + c
                        nc.gpsimd.affine_select(
                            out=halves(pt, 0), in_=halves(pt, 0),
                            pattern=[[0, 2], [1, 512 - off]],
                            compare_op=mybir.AluOpType.is_ge, fill=0.0,
                            base=off, channel_multiplier=-1)
                    else:
                        nc.vector.tensor_mul(halves(pt, 0), halves(pt, 0),
                                             halves(msk2_sb, 1024 * r))
                for h in (0, 1):
                    nc.tensor.matmul(
                        pvt[h][:, off:512],
                        v_sb[:, j * 130 + 65 * h:j * 130 + 65 * h + 65],
                        pt[:, 512 * h + off:512 * h + 512],
                        start=(j == 0), stop=(j == nj - 1))

            # ---- normalize into attnT ----
            for h in (0, 1):
                rc = rcp.tile([1, 512], F32R, tag="rc", name=f"rc{h}_{i}")
                with nc.allow_low_precision(reason="f32r is 4-byte float"):
                    nc.vector.reciprocal(rc[:], pvt[h][64:65, :])
                rct = rcp.tile([64, 512], F32, tag="rct", name=f"rct{h}_{i}")
                if BCAST_MODE == "pool":
                    nc.gpsimd.partition_broadcast(rct[:], rc[:].bitcast(F32))
                else:
                    rctp = scps.tile([64, 512], F32, tag="sc", name=f"rctp{h}_{i}")
                    nc.tensor.matmul(rctp[:], ones1_sb[:], rc[:],
                                     start=True, stop=True)
                    nc.vector.tensor_copy(rct[:], rctp[:])
                nc.vector.tensor_mul(at_sb[64 * h:64 * h + 64, sl],
                                     pvt[h][0:64, :], rct[:])

        LOOKAHEAD = int(os.environ.get("LOOKAHEAD", 1))
        for i in range(NSL):
            emit_qkv(i)
            if i >= LOOKAHEAD:
                emit_attn(i - LOOKAHEAD)
            if WO_MODE == "lastslice" and i == NSL - 1:
                # projections are done; overlap Wo for finished slices with
                # the final (longest) attention slice
                _emit_wo(nc, mmps, tmpp, at_sb, wot_sb, out_d,
                         range(4 * (NSL - LOOKAHEAD)))
        for i in range(NSL - LOOKAHEAD, NSL):
            emit_attn(i)
        if WO_MODE == "lastslice":
            _emit_wo(nc, mmps, tmpp, at_sb, wot_sb, out_d,
                     range(4 * (NSL - LOOKAHEAD), NB))

        if WO_MODE == "tail":
            _emit_wo(nc, mmps, tmpp, at_sb, wot_sb, out_d, range(NB))
        elif WO_MODE == "delayed":
            _emit_wo(nc, mmps, tmpp, at_sb, wot_sb, out_d, range(4 * (NSL - 1), NB))


_CACHE = {}


def _get_nc():
    if "nc" not in _CACHE:
        nc = bacc.Bacc("TRN2", target_bir_lowering=False, debug=False,
                       num_devices=N_CORES)
        with tile.TileContext(nc) as tc:
            _emit(tc)
        nc.compile()
        _CACHE["nc"] = nc
    return _CACHE["nc"]


def _host_tables():
    if "tables" in _CACHE:
        return _CACHE["tables"]
    inv = (1.0 / (ROPE_BASE ** (np.arange(0, HD, 2, dtype=np.float32) / HD))
           ).astype(np.float32)
    ang = np.arange(S, dtype=np.float32)[:, None] * inv[None, :]   # (S, 32)
    cos = np.concatenate([np.cos(ang), np.cos(ang)], axis=1)       # (S, 64)
    sin = np.concatenate([np.sin(ang), np.sin(ang)], axis=1)
    cost = np.ascontiguousarray(
        np.concatenate([cos.T, cos.T], axis=0), dtype=np.float32)  # (128, S)
    sint = np.ascontiguousarray(
        np.concatenate([sin.T, sin.T], axis=0), dtype=np.float32)

    # rot = R @ q per 64-block: rot[p] = -q[p+32] (p%64<32), q[p-32] (else)
    R = np.zeros((128, 128), np.float32)
    for base in (0, 64):
        for p in range(32):
            R[base + p, base + p + 32] = -1.0
            R[base + p + 32, base + p] = 1.0
    rmt = np.ascontiguousarray(R.T)

    k = np.arange(128)[:, None]
    q = np.arange(512)[None, :]
    msk = np.concatenate(
        [np.concatenate([m, m], axis=1)
         for m in ((128 * r + k <= q).astype(np.float32) for r in range(4))],
        axis=1)
    import ml_dtypes
    msk = np.ascontiguousarray(msk).astype(ml_dtypes.bfloat16)     # (128, 4096)

    ones = np.ones((128, 64), np.float32)
    _CACHE["tables"] = (cost, sint, rmt, msk, ones)
    return _CACHE["tables"]


def _in_maps(x, Wq, Wk, Wv, Wo):
    x2 = np.asarray(x, dtype=np.float32).reshape(S, D)
    xt = np.ascontiguousarray(x2.T)
    Wq = np.asarray(Wq, dtype=np.float32)
    Wk = np.asarray(Wk, dtype=np.float32)
    Wv = np.asarray(Wv, dtype=np.float32)
    Wo = np.asarray(Wo, dtype=np.float32)
    cost, sint, rmt, msk, ones = _host_tables()
    maps = []
    for c in range(N_CORES):
        rows = slice(128 * c, 128 * (c + 1))
        maps.append({
            "xt": xt,
            "wqt": np.ascontiguousarray(Wq[rows, :].T),
            "wkt": np.ascontiguousarray(Wk[rows, :].T),
            "wvt": np.ascontiguousarray(Wv[rows, :].T),
            "wot": np.ascontiguousarray(Wo[:, rows].T),
            "cost": cost, "sint": sint, "rmt": rmt, "msk": msk, "ones": ones,
        })
    return maps


def kernel(x, Wq, Wk, Wv, Wo):
    nc = _get_nc()
    maps = _in_maps(x, Wq, Wk, Wv, Wo)
    res = run_bass_kernel_spmd(nc, maps, list(range(N_CORES)))
    acc = np.zeros((S, D), np.float32)
    for c in range(N_CORES):
        acc += res.results[c]["out"]
    return acc.reshape(1, S, D)

